# revision 20
# baseline (speedup 1.0000x reference)
"""DenseGrid multi-resolution 1-D linear interpolation on 8 Trainium2 cores.

Math: out[n, l, f] = (1-fr)*storage[off_l + i0, f] + fr*storage[off_l + i0 + 1, f]
with i0 = floor(x[n]*(R_l-1)), fr = frac(x[n]*(R_l-1)).

v2 design (per core, data-parallel over N):
  The 16 level grids (m = R-1 in {4,6,...,34}) are FUSED onto 9 shared fine
  grids (M in {32,24,20,28,18,22,26,30,34}) wherever levels nest exactly
  (m | M), shrinking the tent basis from 320 to 243 rows with ZERO error:
  a level's piecewise-linear interpolant re-sampled on a nesting fine grid
  is reproduced exactly by the fine grid's tent (hat) basis.

  Pipeline per pair of 512-pt chunks (A, B):
   1. PE (4 concurrent row-band matmuls, K=3): psA[row, n] = M_g*x_n - J
      for the 243 (+13 pad) basis rows, exact via fp16 hi/lo split of x.
   2. DVE custom tent op / ACT Abs+Relu (FD=1024): T = relu(1 - |psA|), fp16.
   3. PE (2 col-band concurrent matmuls, N=512): psO[ch, n] += V_k.T @ T_k
      with the 243x64 fused value table STATIONARY (values change never,
      tents stream) - big N keeps the PE warm at 2.4 GHz.
   4. ACT: psO -> SBUF fp16 (channel-major), DMA out per super-chunk.
  Output is [ch, n] fp16 on device; host reassembles to [N, 16, 4] fp32.
"""

import numpy as np

import concourse.bacc as bacc
import concourse.mybir as mybir
import concourse.tile as tile
from concourse.bass_utils import run_bass_kernel_spmd

# ----------------------------------------------------------------------------
# Problem constants (hardcoded per spec)
# ----------------------------------------------------------------------------
N_FULL = 1_048_576
LEVELS = 16
FEAT = 4
N_CORES = 8
NCP = N_FULL // N_CORES            # points per core = 131072
RESOLUTIONS = [2 * i + 1 for i in range(2, LEVELS + 2)]   # [5,7,...,35]

# Fused tent basis: each group (M, [m...]) serves every level with m | M on
# one fine grid of M+1 knots. Total rows = sum(M+1) = 243.
GROUPS = [
    (32, [4, 8, 16, 32]),
    (24, [6, 12, 24]),
    (20, [10, 20]),
    (28, [14, 28]),
    (18, [18]),
    (22, [22]),
    (26, [26]),
    (30, [30]),
    (34, [34]),
]
KROWS = sum(M + 1 for M, _ in GROUPS)          # 243
P = 128
KPAD = 2 * P                                   # rows padded to 256 (2 chunks)
CHUNK = 512                                    # points per chunk
PAIR = 2 * CHUNK
PAIRS_PER_SUPER = 16                           # output DMA batch (16384 pts)
PAD_J = 60000.0                                # pad rows: psA=-60000 -> tent=0
FILLER_MMS = 2                                 # zero-MMs (HAM warmth), avg 1.5

# ----------------------------------------------------------------------------
# Custom DVE op: tent(v) = relu(1 - |v|)
# ----------------------------------------------------------------------------
_TENT_NAME = "TENT0_ANT_DG"


def _register_tent_op():
    from concourse import dve_ops
    from concourse.dve_spec import Spec, Src0, One, Zero, relu, maxx, lower
    from concourse.dve_table_gen import DveOpSpec

    if any(op.name == _TENT_NAME for op in dve_ops.OPS):
        return next(op for op in dve_ops.OPS if op.name == _TENT_NAME)

    body = relu(One - maxx(Src0, Zero - Src0))
    spec = Spec(
        body=body,
        reference=lambda in0, in1, s0, s1, imm2: np.maximum(
            1.0 - np.abs(np.asarray(in0, np.float32)), 0.0
        ),
    )
    shas = {}
    for ver in ("v3", "v4"):
        s = DveOpSpec(name=_TENT_NAME, opcode=0, uops=lower(spec, ver=ver), rd1_en=False)
        shas[ver] = s.sha(ver)
    op = dve_ops.DveOp(_TENT_NAME, spec, subdim=False, uops_sha=shas)
    dve_ops.OPS.append(op)
    dve_ops._SUB_OPCODE_FOR_NAME[op.name] = (
        dve_ops._CUSTOM_DVE_ROW_BASE + len(dve_ops.OPS) - 1
    )
    dve_ops.CUSTOM_DVE_SPECS[op.name] = op.spec
    return op


# ----------------------------------------------------------------------------
# Host table prep (tiny: 320x4 storage -> fused 243-row basis tables)
# ----------------------------------------------------------------------------
def _row_list():
    """[(g, M, J)] for the 243 basis rows, in group order."""
    rows = []
    for g, (M, _) in enumerate(GROUPS):
        for J in range(M + 1):
            rows.append((g, M, J))
    return rows


def _chan_list():
    """[(l, f)] for the 64 output channels, in group order."""
    chans = []
    for _, ms in GROUPS:
        for m in ms:
            l = (m - 4) // 2
            for f in range(FEAT):
                chans.append((l, f))
    return chans


def make_tables(storage, resolutions):
    storage = np.asarray(storage, np.float64)
    res = np.asarray(resolutions, np.int64)
    offs = np.concatenate([[0], np.cumsum(res)[:-1]])
    rows = _row_list()
    assert len(rows) == KROWS

    # Fused value table: Cval[row, ch] = level-l interpolant at knot J/M.
    cval = np.zeros((KPAD, 64), np.float64)
    ch = 0
    for g, (M, ms) in enumerate(GROUPS):
        row0 = sum(GROUPS[gg][0] + 1 for gg in range(g))
        for m in ms:
            l = (m - 4) // 2
            assert res[l] - 1 == m and M % m == 0
            for J in range(M + 1):
                s = J * m                      # position * M (integer)
                i0, rem = divmod(s, M)
                v = storage[offs[l] + i0]
                if rem:
                    v = ((M - rem) * v + rem * storage[offs[l] + i0 + 1]) / M
                cval[row0 + J, ch : ch + FEAT] = v
            ch += FEAT
    assert ch == 64

    # Affine stationaries (M, M, -J) per row; pad rows -> tent = 0.
    mst = np.zeros((3, KPAD), np.float64)
    for r, (g, M, J) in enumerate(rows):
        mst[0, r] = M
        mst[1, r] = M
        mst[2, r] = -J
    for r in range(KROWS, KPAD):
        mst[2, r] = -PAD_J

    # mstat tile [99, 128]: chunk0 rows at partitions 0-2 / 64-66,
    # chunk1 rows at partitions 32-34 / 96-98.
    mstat = np.zeros((99, P), np.float16)
    mstat[0:3, :] = mst[:, 0:P].astype(np.float16)
    mstat[64:67, :] = mst[:, 0:P].astype(np.float16)
    mstat[32:35, :] = mst[:, P:KPAD].astype(np.float16)
    mstat[96:99, :] = mst[:, P:KPAD].astype(np.float16)

    # value tile [128, 256]: k0 | k1 | k0-copy | k1-copy (copies feed the
    # col-band-B matmuls; content identical, tile_position sets the band).
    val = np.zeros((P, 5 * 64), np.float16)   # 5th block: zeros (filler MMs)
    val[:, 0:64] = cval[0:P].astype(np.float16)
    val[:, 64:128] = cval[P:KPAD].astype(np.float16)
    val[:, 128:192] = val[:, 0:64]
    val[:, 192:256] = val[:, 64:128]
    return mstat, val


# ----------------------------------------------------------------------------
# Bass program (SPMD, one program for all cores)
# ----------------------------------------------------------------------------
def build_program(ncp=NCP, pairs_per_super=PAIRS_PER_SUPER):
    tent_op = _register_tent_op()
    n_pairs = ncp // PAIR
    pairs_per_super = min(pairs_per_super, n_pairs)
    sup_pts = pairs_per_super * PAIR           # 16384
    sup_cols = sup_pts // 2                    # output cols per super = 8192

    f32 = mybir.dt.float32
    f16 = mybir.dt.float16
    AF = mybir.ActivationFunctionType

    nc = bacc.Bacc()
    x_ext = nc.declare_dram_parameter("x", [3, ncp], f16, isOutput=False)
    mstat_ext = nc.declare_dram_parameter("mstat", [99, P], f16, isOutput=False)
    val_ext = nc.declare_dram_parameter("val", [P, 5 * 64], f16, isOutput=False)
    out_ext = nc.declare_dram_parameter("out", [P, ncp // 2], f16, isOutput=True)

    n_steps = ncp // CHUNK
    steps_per_super = sup_pts // CHUNK

    with tile.TileContext(nc) as tc:
        with (
            tc.tile_pool(name="consts", bufs=1) as cpool,
            tc.tile_pool(name="xin", bufs=2) as xpool,
            tc.tile_pool(name="tent", bufs=3) as tpool,
            tc.tile_pool(name="obuf", bufs=2) as opool,
            tc.tile_pool(name="psA", bufs=3, space="PSUM") as psa_pool,
            tc.tile_pool(name="psO", bufs=2, space="PSUM") as pso_pool,
        ):
            mstat_t = cpool.tile([99, P], f16, tag="mstat")
            val_t = cpool.tile([P, 5 * 64], f16, tag="val")
            nc.sync.dma_start(out=mstat_t[:], in_=mstat_ext[:])
            nc.sync.dma_start(out=val_t[:], in_=val_ext[:])

            x_ts = {}
            o_ts = {}
            ts = {}
            psos = {}

            def load_x(s):
                x_t = xpool.tile([99, sup_pts], f16, tag="x", name=f"x_{s}")
                half = sup_pts // 2
                for h in range(2):
                    for rb in (0, 32, 64, 96):
                        nc.sync.dma_start(
                            out=x_t[rb : rb + 3, h * half : (h + 1) * half],
                            in_=x_ext[
                                :,
                                s * sup_pts + h * half : s * sup_pts + (h + 1) * half,
                            ],
                        )
                x_ts[s] = x_t

            def emit_front(p):
                """Affines + tent for step p (PE row bands alternate parity)."""
                s, lp = divmod(p, steps_per_super)
                if lp == 0:
                    if s == 0:
                        load_x(0)
                    o_ts[s] = opool.tile([P, sup_cols], f16, tag="o", name=f"o_{s}")
                if lp == steps_per_super // 2 and (p + steps_per_super) < n_steps:
                    load_x(s + 1)   # prefetch next super's x mid-super
                x_t = x_ts[s]
                odd = p % 2
                xs = slice(lp * CHUNK, (lp + 1) * CHUNK)
                rbs = (64, 96) if odd else (0, 32)
                pa = psa_pool.tile([P, 1024], f32, tag="c", name=f"psa_{p}")
                for k, rb in enumerate(rbs):
                    nc.tensor.matmul(
                        pa[:, k * CHUNK : (k + 1) * CHUNK],
                        lhsT=mstat_t[rb : rb + 3, :],
                        rhs=x_t[rb : rb + 3, xs],
                        start=True,
                        stop=True,
                        tile_position=(rb, 0),
                    )
                T = tpool.tile([P, 1024], f16, tag="T", name=f"T_{p}")
                if p % 4 < 3:
                    nc.vector._custom_dve(tent_op, out=T[:], in0=pa[:])
                else:
                    nc.scalar.activation(T[:], pa[:], AF.Abs)
                    nc.scalar.activation(T[:], T[:], AF.Relu, bias=1.0, scale=-1.0)
                ts[p] = T

            def emit_mains(p):
                s, lp = divmod(p, steps_per_super)
                T = ts.pop(p)
                odd = p % 2
                if not odd:
                    psos[p // 2] = pso_pool.tile(
                        [P, CHUNK], f32, tag="O", name=f"psO_{p}"
                    )
                psO = psos[p // 2]
                cb = 64 * odd
                o_sl = psO[cb : cb + 64, :]
                vc = 128 * odd
                nc.tensor.matmul(
                    o_sl, lhsT=val_t[:, vc : vc + 64], rhs=T[:, 0:CHUNK],
                    start=True, stop=False, tile_position=(0, cb), skip_group_check=True,
                )
                for _ in range(FILLER_MMS - (p % 2)):
                    # zero-stationary matmul: psO += 0; keeps the PE array busy
                    # so the HAM clock gate stays at full rate through stalls
                    nc.tensor.matmul(
                        o_sl, lhsT=val_t[:, 256:320], rhs=T[:, 0:CHUNK],
                        start=False, stop=False, tile_position=(0, cb),
                        skip_group_check=True,
                    )
                nc.tensor.matmul(
                    o_sl, lhsT=val_t[:, vc + 64 : vc + 128], rhs=T[:, CHUNK:PAIR],
                    start=False, stop=True, tile_position=(0, cb), skip_group_check=True,
                )
                if odd:
                    t2 = lp // 2
                    nc.scalar.copy(
                        o_ts[s][:, t2 * CHUNK : (t2 + 1) * CHUNK],
                        psos.pop(p // 2)[:],
                    )
                if lp == steps_per_super - 1:
                    nc.sync.dma_start(
                        out=out_ext[:, s * sup_cols : (s + 1) * sup_cols],
                        in_=o_ts[s][:],
                    )

            for p in range(n_steps):
                emit_front(p)
                if p >= 2:
                    emit_mains(p - 2)
            emit_mains(n_steps - 2)
            emit_mains(n_steps - 1)
    nc.finalize()
    return nc


# ----------------------------------------------------------------------------
# Host entry point
# ----------------------------------------------------------------------------
def _prep_x(x_shard):
    """Lossless fp16 hi/lo split of fp32 x, plus a ones row (K=3 affine)."""
    xh = x_shard.astype(np.float16)
    xl = (x_shard - xh.astype(np.float32)).astype(np.float16)
    ones = np.ones_like(xh)
    return np.stack([xh, xl, ones])


_PROGRAM_CACHE = {}
LAST_RESULT = None


def kernel(x, storage, resolutions, _trace=False):
    global LAST_RESULT
    x = np.asarray(x, np.float32).reshape(-1)
    assert x.shape[0] == N_FULL
    mstat, val = make_tables(storage, resolutions)

    if NCP not in _PROGRAM_CACHE:
        _PROGRAM_CACHE[NCP] = build_program(NCP)
    nc = _PROGRAM_CACHE[NCP]

    in_maps = []
    for c in range(N_CORES):
        shard = x[c * NCP : (c + 1) * NCP]
        in_maps.append({"x": _prep_x(shard), "mstat": mstat, "val": val})
    res = run_bass_kernel_spmd(nc, in_maps, list(range(N_CORES)), trace=_trace)
    LAST_RESULT = res

    # channel permutation: chans[ci] = (l, f) -> column order for [l*4+f]
    chans = _chan_list()
    pos = np.zeros(64, np.int64)
    for ci, (l, f) in enumerate(chans):
        pos[l * FEAT + f] = ci

    outs = []
    for r in res.results:
        ext = np.asarray(r["out"])                      # [128, NCP//2] fp16
        oc = ext.reshape(P, NCP // PAIR, CHUNK)
        ab = np.stack([oc[0:64], oc[64:128]], axis=2)   # [64, pairs, 2, 512]
        chmaj = ab.reshape(64, NCP)
        outs.append(chmaj.T[:, pos].astype(np.float32).reshape(NCP, LEVELS, FEAT))
    return np.concatenate(outs, axis=0)
